# revision 10
# baseline (speedup 1.0000x reference)
"""Trainium2 Bass kernel for nn_AttentionNeNode (8-core SPMD).

Math being computed (see problem reference):
    sel  = inputs[:, in_idxs]            # [R, L] column gather
    qkv  = sel @ weights                 # [R, 3] -> q, k, v columns
    out  = sigmoid(softmax(q[-1] * k.T) @ v)   # only the LAST row's attention matters

Key transformations:
  1. Column gather + matmul == dense matmul with scattered weights:
         sel @ weights == inputs @ W_dense,
     where W_dense[f] = sum of weights[l] over l with in_idxs[l] == f.
  2. Only the UNIQUE gathered columns matter (W_dense is zero elsewhere), so
     the host packs just those columns (~1620 of 4096) before shipping to the
     device: 2.5x less HBM traffic, numerically exact.
  3. The packed activations stream in fp8e4m3 (4x fewer bytes than f32). The
     PE runs four concurrent column-tiled matmuls (tile_position=(0,32g),
     one 256-row group each) so the array consumes ~4 rhs columns/cycle and
     keeps up with the DMA stream.
  4. The device computes k,v per row; the host does the exact flash-softmax
     combine in f64, re-computing k,v from the original f32 data for the
     handful of rows whose logits are within a safety margin of the max, so
     device precision cannot affect the final answer.
  5. inputs are pre-transposed/tiled on host so the contraction dim lands on
     SBUF partitions and DMA descriptors are large contiguous runs.
"""

import sys

if "/opt/trn_rl_repo" not in sys.path:
    sys.path.insert(0, "/opt/trn_rl_repo")

import numpy as np
import ml_dtypes

import concourse.bacc as bacc
import concourse.tile as tile
from concourse import mybir
from concourse.bass_utils import run_bass_kernel_spmd

R, F = 8192, 4096
NCORES = 8
RB = R // NCORES            # 1024 rows per core
NG = 4                      # concurrent PE column-tile groups
GR = RB // NG               # 256 rows per group (psum free dim)
FP8 = mybir.dt.float8e4
F32 = mybir.dt.float32
NP_FP8 = ml_dtypes.float8_e4m3
# margin (in logit units) below the max logit within which rows are exactly
# re-computed on the host; fp8 logit error std is ~|q| so this covers >30
# sigma while keeping the candidate set tiny for well-separated maxima.
CAND_MARGIN_Q = 40.0

_NC_CACHE = {}


def _build_nc(nch):
    nc = bacc.Bacc("TRN2", target_bir_lowering=False, debug=False)
    xt = nc.dram_tensor("xt", [128, nch, RB], FP8, kind="ExternalInput").ap()
    wsb = nc.dram_tensor("wsb", [128, 2 * nch], FP8, kind="ExternalInput").ap()
    out = nc.dram_tensor("out", [98, GR], mybir.dt.bfloat16,
                         kind="ExternalOutput").ap()

    # chunk-granularity DMA tiles: first and last are single chunks so the
    # PE starts early and almost no matmul work remains after the last byte
    # lands (tile-granularity semaphores gate the PE on whole tiles)
    def tiles_of(n):
        head, tail = [1], [1]
        n -= 2
        mid = []
        while n > 0:
            t = min(3, n)
            mid.append(t)
            n -= t
        return head + mid + tail

    splits = tiles_of(nch)

    with tile.TileContext(nc) as tc:
        with tc.tile_pool(name="consts", bufs=1) as consts, \
             tc.tile_pool(name="xtiles", bufs=len(splits)) as xtiles, \
             tc.tile_pool(name="ps", bufs=1, space="PSUM") as psp, \
             tc.tile_pool(name="tail", bufs=1) as tailp:
            # weight load rides the scalar HWDGE ring (free: nothing else
            # uses it until the final store), x tiles stream on sync
            w_t = consts.tile([128, 2 * nch], FP8)
            nc.scalar.dma_start(out=w_t[:], in_=wsb)
            x_ts = []
            c0 = 0
            for i, nt in enumerate(splits):
                x_t = xtiles.tile([128, nt, RB], FP8, tag="x_t")
                eng = nc.sync if i % 2 == 0 else nc.scalar
                eng.dma_start(out=x_t[:], in_=xt[:, c0:c0 + nt, :])
                x_ts.append((x_t, c0, nt))
                c0 += nt

            # k,v accumulate in one PSUM bank: group g owns partitions
            # {32g, 32g+1} and rows g*GR..(g+1)*GR-1, so four column-tiled
            # matmuls run concurrently per chunk
            ps_kv = psp.tile([98, GR], F32)
            for x_t, c0, nt in x_ts:
                for u in range(nt):
                    c = c0 + u
                    st, sp = (c == 0), (c == nch - 1)
                    for g in range(NG):
                        nc.tensor.matmul(
                            ps_kv[32 * g:32 * g + 2, :],
                            w_t[:, 2 * c:2 * c + 2],
                            x_t[:, u, g * GR:(g + 1) * GR],
                            start=st, stop=sp, tile_position=(0, 32 * g),
                            skip_group_check=True)
            # evacuate PSUM (single wide DVE copy; unused partitions ride
            # along) and ship the raw k,v rows
            kv_sb = tailp.tile([98, GR], mybir.dt.bfloat16)
            nc.vector.tensor_scalar_add(out=kv_sb[:], in0=ps_kv[:],
                                        scalar1=0.0)
            nc.scalar.dma_start(out=out, in_=kv_sb[:])
    nc.finalize()
    return nc


def _get_nc(nch):
    if nch not in _NC_CACHE:
        _NC_CACHE[nch] = _build_nc(nch)
    return _NC_CACHE[nch]


def _prep_inputs(inputs, in_idxs, weights):
    inputs = np.ascontiguousarray(np.asarray(inputs, dtype=np.float32))
    idx = np.asarray(in_idxs).astype(np.int64)
    w = np.asarray(weights, dtype=np.float32)

    # scatter-add weights onto the UNIQUE gathered columns:
    # sel @ weights == inputs[:, uniq] @ wu
    uniq, inv = np.unique(idx, return_inverse=True)
    nu = len(uniq)
    wu = np.zeros((nu, 3), dtype=np.float64)
    np.add.at(wu, inv, w.astype(np.float64))

    nch = (nu + 127) // 128
    fpad = nch * 128

    # packed activation block [R, fpad] in fp8 (zero-padded features)
    a = np.zeros((R, fpad), dtype=NP_FP8)
    a[:, :nu] = inputs[:, uniq].astype(NP_FP8)
    wpad = np.zeros((fpad, 3), dtype=np.float64)
    wpad[:nu] = wu

    # wsb[p, 2c+m] = wpad[c*128 + p, 1+m]  (k and v weight columns)
    wsb = np.ascontiguousarray(
        wpad[:, 1:3].astype(np.float32).astype(NP_FP8)
        .reshape(nch, 128, 2).transpose(1, 0, 2).reshape(128, 2 * nch))

    # xt[core][p, c, r] = a[core*RB + r, c*128 + p]
    x4 = a.reshape(NCORES, RB, nch, 128)
    xt_all = np.ascontiguousarray(x4.transpose(0, 3, 2, 1))

    in_maps = [{"xt": xt_all[i], "wsb": wsb} for i in range(NCORES)]
    host_ctx = {
        "inputs": inputs, "uniq": uniq, "wu": wu, "nch": nch,
        # exact last-row q in f64 (one tiny dot product)
        "q_last": float(inputs[R - 1, uniq].astype(np.float64) @ wu[:, 0]),
    }
    return in_maps, host_ctx


def _combine(kv, host_ctx):
    # kv: [NCORES, 98, GR]; group g of core i holds k at partition 32g and v
    # at partition 32g+1, for rows i*RB + g*GR ... Exact f64 flash-softmax
    # with host-side exact recompute of every row whose logit is within the
    # safety margin of the max.
    kv = np.asarray(kv, dtype=np.float64)
    k_dev = kv[:, 0:98:32, :].reshape(R)
    v_dev = kv[:, 1:98:32, :].reshape(R)
    q = host_ctx["q_last"]
    x = q * k_dev
    margin = CAND_MARGIN_Q * max(abs(q), 1.0) + 40.0
    cand = np.nonzero(x >= x.max() - margin)[0]
    # exact k,v for candidate rows from the original f32 data
    a_c = host_ctx["inputs"][cand][:, host_ctx["uniq"]].astype(np.float64)
    kv_c = a_c @ host_ctx["wu"][:, 1:3]
    x[cand] = q * kv_c[:, 0]
    v = v_dev
    v[cand] = kv_c[:, 1]
    m = x.max()
    e = np.exp(x - m)
    val = (e * v).sum() / e.sum()
    if val >= 0:
        sig = 1.0 / (1.0 + np.exp(-val))
    else:
        ev = np.exp(val)
        sig = ev / (1.0 + ev)
    return np.array([[sig]], dtype=np.float32)


def kernel(inputs, in_idxs, weights):
    in_maps, host_ctx = _prep_inputs(inputs, in_idxs, weights)
    nc = _get_nc(host_ctx["nch"])
    res = run_bass_kernel_spmd(nc, in_maps, core_ids=list(range(NCORES)))
    kv = np.stack([res.results[i]["out"] for i in range(NCORES)])
    return _combine(kv, host_ctx)


if __name__ == "__main__":
    rng = np.random.default_rng(0)
    inputs = rng.standard_normal((R, F), dtype=np.float32)
    in_idxs = rng.integers(0, F, size=2048)
    weights = rng.standard_normal((2048, 3), dtype=np.float32)
    got = kernel(inputs, in_idxs, weights)
    sel = inputs[:, in_idxs]
    qkv = sel.astype(np.float64) @ weights.astype(np.float64)
    q, k, v = qkv[:, 0], qkv[:, 1], qkv[:, 2]
    logits = q[-1] * k
    a = np.exp(logits - logits.max())
    want = a @ v / a.sum()
    want = 1.0 / (1.0 + np.exp(-want))
    print("got", got, "want", want,
          "relerr", abs(got[0, 0] - want) / max(abs(want), 1e-30))
